# revision 16
# baseline (speedup 1.0000x reference)
"""CrossAttention kernel for 8 TRN2 NeuronCores.

Sharding (Megatron head-parallel): core c owns heads {2c, 2c+1} = output
channels [128c, 128c+128).
  - column-parallel q/k/v projections (full activations in, per-core head
    channels out)
  - full attention for the core's heads (both batch elements)
  - row-parallel out projection -> partial [4096, 1024] fp32; host sums the
    8 partials. v-bias and out-bias fold exactly into the host-side
    epilogue: out = sum_c partial_c + Wo @ bv + bo (softmax rows sum to 1).

Per-core dataflow (t = b*2048 + n, 4096 tokens):
  pass0 (per 1024-token chunk): pj [128ch, 512] = W @ xT chains, evicted in
    ONE [128,512] tensor_scalar_add to qT/kT [128ch, T] (rows 0:64 = head A
    d-dims, 64:128 = head B -- the natural layout, no duplication).
    v [t, ch] chains -> v_aug[u] [128 j, 16 jt, 130] (cols 65h+64 are ones).
  attention block (u, iw in 0..3, BOTH heads, i-window 512): 16 j-steps:
    scores: sp[:, 0:512]   = kT[0:64, j].T   @ qT[0:64, iw]    (head A)
            sp[:, 512:1024]= kT[64:128, j].T @ qT[64:128, iw]  (head B)
            -- the two matmuls use disjoint PE row groups, run concurrently
    exp:    pexp [128, 1024] = exp(0.125 * sp)   (ACT, PSUM->SBUF fp16)
    ctx:    cps[:, 0:512]   += v_aug[., jt, 0:65].T   @ pexp[:, 0:512]
            cps[:, 512:1024]+= v_aug[., jt, 65:130].T @ pexp[:, 512:1024]
            (row 64 of each half accumulates the softmax denominator r[i])
  normalize: one DVE copy cps->ctx_s [65,1024]; reciprocal_approx_fast on
    row 64; gpsimd partition_broadcast; 2 DVE muls -> ctxT[u] fp16.
  outproj: po [128t, 512] = ctxT[u][:, tt].T @ wo half; evict fp16, DMA out.

Projection / out-proj work units are interleaved between attention j-steps
so the PE fills the slack under the ACT-bound softmax exp stream.
PSUM banks: sps0/sps1 [128,1024] (2+2), cps [65,1024] (2), pj/po [128,512]
(1+1) = 8.
"""

import numpy as np
from contextlib import ExitStack

import concourse.bass as bass
import concourse.tile as tile
from concourse import bacc, mybir
from concourse.bass_utils import run_bass_kernel_spmd

AF = mybir.ActivationFunctionType

# ---- problem constants (hardcoded per contract) ----
B, N, C, H, D = 2, 2048, 1024, 16, 64
T = B * N            # 4096 flattened tokens
CH = 128             # channels per core = 2 heads * 64
NCORES = 8
SCALE = D ** -0.5    # 0.125

# ---- tunables ----
DT = mybir.dt.float16      # on-chip matmul/storage dtype
NPDT = np.float16
TCH = 1024                 # pass0 t-chunk
NTCH = T // TCH            # 4
FT = C // 128              # 8 f-tiles (contraction tiles for projections)
IW = 512                   # attention i-window per block
JT = 128                   # j tile (kv) size
NJT = N // JT              # 16


def emit(tc: tile.TileContext, aps: dict):
    nc = tc.nc
    ctx = ExitStack()
    with ctx:
        const = ctx.enter_context(tc.tile_pool(name="const", bufs=1))
        persist = ctx.enter_context(tc.tile_pool(name="persist", bufs=1))
        xpool = ctx.enter_context(tc.tile_pool(name="xpool", bufs=2))
        ppool = ctx.enter_context(tc.tile_pool(name="ppool", bufs=2))
        rpool = ctx.enter_context(tc.tile_pool(name="rpool", bufs=2))
        opool = ctx.enter_context(tc.tile_pool(name="opool", bufs=4))
        psum = ctx.enter_context(tc.tile_pool(name="psum", bufs=1, space="PSUM"))

        # ---- load weights (wq first: the q chain of chunk 0 runs first) ----
        w_sb = {name: const.tile([128, FT, CH], DT, name=name, tag=name)
                for name in ("wq", "wk", "wv")}
        wo_sb = const.tile([128, C], DT, tag="wo")
        bias_sb = const.tile([128, 2], mybir.dt.float32, tag="bias")

        def load_weights(names):
            # weights go on the ACT dispatch queue (hwdge) so the SP queue
            # starts streaming activations immediately; the rings interleave
            # the transfers and these are small
            for name in names:
                nc.scalar.dma_start(w_sb[name][:],
                                    aps[name].rearrange("(f p) m -> p f m", p=128))

        # ---- persistent activations ----
        # qT/kT [128 ch, T]: rows 0:64 head A dims, rows 64:128 head B
        qT = persist.tile([128, T], DT, name="qT", tag="qT")
        kT = persist.tile([128, T], DT, name="kT", tag="kT")
        # v_aug[u]: [128 j, 16 jt, 130]; cols 65h+64 are ones
        v_aug = [persist.tile([128, NJT, 130], DT, name=f"vaug{u}", tag=f"vaug{u}")
                 for u in range(B)]
        for u in range(B):
            for h in range(2):
                nc.vector.memset(v_aug[u][:, :, 65 * h + 64:65 * h + 65], 1.0)
        # ctxT[u]: [128 ch, 2048 i] normalized context^T (head B rows 64-127)
        ctxT = [persist.tile([128, N], DT, name=f"ctxT{u}", tag=f"ctxT{u}")
                for u in range(B)]

        xr = {k: aps[k].rearrange("(f p) t -> p f t", p=128) for k in ("xq", "xk", "xv")}

        # pass0 / outproj share the two 1-bank psum slots pj & po; pass0 is
        # done (chunk 3 consumed during block 3) before outproj starts
        # (block 4), so phases never contend.
        pingpong = [0]

        def small_ps(name):
            tag = ("pj", "po")[pingpong[0] % 2]
            pingpong[0] += 1
            return psum.tile([128, 512], mybir.dt.float32, name=name, tag=tag)

        # ---------------- pass0: projections for one 1024-token chunk -----
        # returns (dma_fn, [compute units]); the driver kicks the DMA a
        # block ahead of when the compute units get filled in.
        def pass0_units(tch):
            gen = []
            ts = slice(tch * TCH, (tch + 1) * TCH)
            xq_t = xpool.tile([128, FT, TCH], DT, tag="xq")
            xk_t = xpool.tile([128, FT, TCH], DT, tag="xk")
            xv_t = xpool.tile([128, FT, TCH], DT, tag="xv")

            order = ([("xq", xq_t), ("xk", xk_t), ("xv", xv_t)] if tch == 0
                     else [("xk", xk_t), ("xv", xv_t), ("xq", xq_t)])

            def u_dma():
                if tch == 0:
                    load_weights(["wq", "wk", "wv"])
                    nc.scalar.dma_start(wo_sb[:], aps["wo"])
                for nm, t in order:
                    nc.sync.dma_start(t[:], xr[nm][:, :, ts])

            def qk_half(x_t, wname, bcol, dst, half):
                # two sub-units sharing one psum accumulator: finer fill
                # granularity keeps per-jt PE slack small
                hs = slice(half * 512, (half + 1) * 512)
                gts = slice(tch * TCH + half * 512,
                            tch * TCH + half * 512 + 512)
                state = {}

                def unit_a():
                    pj = small_ps(f"c{tch}{wname}{half}")
                    state["pj"] = pj
                    for ft in range(4):
                        nc.tensor.matmul(pj[:], w_sb[wname][:, ft],
                                         x_t[:, ft, hs],
                                         start=(ft == 0), stop=False)

                def unit_b():
                    pj = state["pj"]
                    for ft in range(4, FT):
                        nc.tensor.matmul(pj[:], w_sb[wname][:, ft],
                                         x_t[:, ft, hs],
                                         start=False, stop=(ft == FT - 1))
                    nc.vector.tensor_scalar_add(
                        dst[:, gts], pj[:], bias_sb[:, bcol:bcol + 1])
                return [unit_a, unit_b]

            def v_half(vg, t4pair):
                # one 512-col psum with two t4 sub-chains (t4pair=0 -> t4 0,1)
                def unit():
                    pv = small_ps(f"c{tch}v{vg}{t4pair}")
                    pv4 = pv[:, 0:256].rearrange("p (t4 hh d) -> p t4 hh d",
                                                 t4=2, d=64)
                    for t4i in range(2):
                        t4 = t4pair * 2 + t4i
                        cs = slice(t4i * 128, (t4i + 1) * 128)
                        gcs = slice(vg * 512 + t4 * 128, vg * 512 + t4 * 128 + 128)
                        for ft in range(FT):
                            nc.tensor.matmul(pv[:, cs], xv_t[:, ft, gcs],
                                             w_sb["wv"][:, ft],
                                             start=(ft == 0), stop=(ft == FT - 1))
                    tt0 = tch * 8 + vg * 4 + t4pair * 2
                    u, jt0 = tt0 // 16, tt0 % 16
                    for h in range(2):
                        nc.vector.tensor_copy(
                            v_aug[u][:, jt0:jt0 + 2, 65 * h:65 * h + 64],
                            pv4[:, :, h])
                return unit

            q_units = [un for hf in range(2)
                       for un in qk_half(xq_t, "wq", 0, qT, hf)]
            k_units = [un for hf in range(2)
                       for un in qk_half(xk_t, "wk", 1, kT, hf)]
            v_units = [v_half(vg, tp) for vg in range(2) for tp in range(2)]
            if tch == 0:
                gen.extend(q_units + k_units + v_units)
            else:
                # k/v first: later j-steps of in-flight blocks need them
                gen.extend(k_units + v_units + q_units)
            return u_dma, gen

        # ---------------- out-proj for one 128-token tile ------------------
        oev = [0]

        def outproj_unit(u, tt):
            def unit():
                osb = opool.tile([128, C], DT, tag="osb")
                for oc in range(2):
                    po = small_ps(f"o{u}t{tt}{oc}")
                    nc.tensor.matmul(
                        po[:], ctxT[u][:, tt * 128:(tt + 1) * 128],
                        wo_sb[:, oc * 512:(oc + 1) * 512],
                        start=True, stop=True)
                    nc.vector.tensor_copy(osb[:, oc * 512:(oc + 1) * 512],
                                          po[:])
                    oev[0] += 1
                nc.sync.dma_start(
                    aps["out"][u * N + tt * 128:u * N + (tt + 1) * 128, :],
                    osb[:])
            return unit

        # ------------- attention block (u, iw): both heads, 16 j-steps ----
        def attention_block(u, iw, fill):
            i0 = u * N + iw * IW
            cps = psum.tile([65, 1024], mybir.dt.float32, name=f"cps{u}{iw}",
                            tag="cps")
            sps = [None, None]
            pexp = [None, None]

            def scores(jt):
                j0 = u * N + jt * JT
                sp = psum.tile([128, 1024], mybir.dt.float32,
                               name=f"sps{jt % 2}", tag=f"sps{jt % 2}")
                nc.tensor.matmul(sp[:, 0:512], kT[0:64, j0:j0 + JT],
                                 qT[0:64, i0:i0 + IW], start=True, stop=True)
                nc.tensor.matmul(sp[:, 512:1024], kT[64:128, j0:j0 + JT],
                                 qT[64:128, i0:i0 + IW], start=True, stop=True)
                sps[jt % 2] = sp

            def expstep(jt):
                pe = ppool.tile([128, 1024], DT, tag=f"pexp{jt % 2}")
                nc.scalar.activation(pe[:], sps[jt % 2][:], AF.Exp, scale=SCALE)
                pexp[jt % 2] = pe

            def ctxstep(jt):
                pe = pexp[jt % 2]
                st = dict(start=(jt == 0), stop=(jt == NJT - 1))
                nc.tensor.matmul(cps[:, 0:512], v_aug[u][:, jt, 0:65],
                                 pe[:, 0:512], **st)
                nc.tensor.matmul(cps[:, 512:1024], v_aug[u][:, jt, 65:130],
                                 pe[:, 512:1024], **st)

            scores(0)
            expstep(0)
            for jt in range(NJT):
                if jt + 1 < NJT:
                    scores(jt + 1)
                    expstep(jt + 1)
                ctxstep(jt)
                # no fills near the block boundary: lets the DVE queue
                # drain so the ctx_s eviction frees cps without delay
                if jt < NJT - 2:
                    fill()
            # normalize: one DVE copy frees the psum accumulator (next
            # block's ctx j-step 0 reuses it); then pull the denominator
            # row to a partition-0 tile with a native DVE copy (custom-ucode
            # ops ignore input partition offsets on HW!), 1/r (fast approx),
            # broadcast, scale. All off the ACT queue: ACT is the exp-stream
            # bottleneck.
            ctx_s = rpool.tile([65, 1024], mybir.dt.float32, tag="ctx_s")
            nc.vector.tensor_copy(ctx_s[:], cps[:])
            rrow = rpool.tile([1, 1024], mybir.dt.float32, name="rrow",
                              tag="rrow")
            nc.vector.tensor_copy(rrow[:], ctx_s[64:65, :])
            rinv = rpool.tile([1, 1024], mybir.dt.float32, name="rinv",
                              tag="rinv")
            nc.vector.reciprocal_approx_fast(out=rinv[:], in_=rrow[:])
            rb = rpool.tile([64, 1024], mybir.dt.float32, name="rb", tag="rb")
            nc.gpsimd.partition_broadcast(rb[:], rinv[:])
            for h in range(2):
                nc.vector.tensor_mul(
                    ctxT[u][64 * h:64 * h + 64, iw * IW:(iw + 1) * IW],
                    ctx_s[0:64, 512 * h:512 * h + 512],
                    rb[:, 512 * h:512 * h + 512])

        # ---------------- driver ------------------------------------------
        # chunk 0: weights on the ACT queue + chunk-0/1 x streams upfront;
        # chunk-0 compute runs inline. Chunk DMAs for c2/c3 are kicked a
        # block ahead of their compute units.
        nc.scalar.dma_start(bias_sb[:], aps["bias"])
        dma1, units1 = pass0_units(1)
        dma2, units2 = pass0_units(2)
        dma3, units3 = pass0_units(3)
        dma0, units0 = pass0_units(0)
        dma0()
        dma1()
        for unit in units0:
            unit()

        # fill schedule: one list of 14 slots (j-steps 0..13) per block,
        # placed so a unit's slot lands at/after its input DMA eta — a
        # DMA-gated fill at the head of the in-order PE queue stalls the
        # whole attention stream. units order for tch>0: k(4) v(4) q(4).
        def grp(units, a, b):
            return list(units[a:b])

        NO = [None]
        op0 = [outproj_unit(0, tt) for tt in range(16)]
        op1 = [outproj_unit(1, tt) for tt in range(16)]
        u1k, u1v = grp(units1, 0, 4), grp(units1, 4, 8)
        sched = [
            # b0: chunk-1 k/v feed THIS block's j-steps: k half covering
            # j-tiles 8-11 must emit before scores(8) (slot<=6), v unit for
            # j-tiles 2i..2i+1 before ctx(2i) (slot<=2i-1); lining them up
            # at their DMA etas too
            NO * 3 + [u1k[0], u1k[1], u1k[2], u1k[3],
                      u1v[0], None, u1v[1], None, u1v[2], None, u1v[3]],
            # b1: q1 ready; k2/v2 arrive mid-block
            grp(units1, 8, 12) + grp(units2, 0, 4) + grp(units2, 4, 6)
            + op0[0:2] + NO * 2,
            # b2: rest of v2, q2, k3 arriving
            grp(units2, 6, 8) + grp(units2, 8, 12) + grp(units3, 0, 4)
            + op0[2:4] + NO * 2,
            # b3: v3, q3, u0 out-proj now flowing
            grp(units3, 4, 8) + grp(units3, 8, 12) + op0[4:10] + NO * 2,
            op0[10:16] + NO * 8,
            op1[0:4] + NO * 10,
            op1[4:8] + NO * 10,
            op1[8:12] + NO * 10,
        ]

        blocks = [(0, 0), (0, 1), (0, 2), (0, 3),
                  (1, 0), (1, 1), (1, 2), (1, 3)]
        for bi, (u, iw) in enumerate(blocks):
            if bi == 1:
                dma2()
            elif bi == 2:
                dma3()
            slots = list(sched[bi])

            def fill(slots=slots):
                if slots:
                    un = slots.pop(0)
                    if un is not None:
                        un()
            attention_block(u, iw, fill)
        # tail out-proj: attention psum banks are free now — use the big
        # [128,1024] slots, and alternate evictions between the now-idle
        # ACT engine and DVE so consecutive tiles fully pipeline
        for i, tt in enumerate(range(12, N // 128)):
            ops = psum.tile([128, 1024], mybir.dt.float32, name=f"ot{tt}",
                            tag=f"sps{i % 2}")
            for oc in range(2):
                nc.tensor.matmul(
                    ops[:, oc * 512:(oc + 1) * 512],
                    ctxT[1][:, tt * 128:(tt + 1) * 128],
                    wo_sb[:, oc * 512:(oc + 1) * 512],
                    start=True, stop=True)
            osb = opool.tile([128, C], DT, tag="osb")
            if i % 2 == 0:
                nc.scalar.copy(osb[:], ops[:])
            else:
                nc.vector.tensor_copy(osb[:], ops[:])
            nc.sync.dma_start(
                aps["out"][N + tt * 128:N + (tt + 1) * 128, :], osb[:])


def build():
    nc = bacc.Bacc("TRN2", target_bir_lowering=False, debug=False)
    aps = {
        "xq": nc.dram_tensor("xq", [C, T], DT, kind="ExternalInput").ap(),
        "xk": nc.dram_tensor("xk", [C, T], DT, kind="ExternalInput").ap(),
        "xv": nc.dram_tensor("xv", [C, T], DT, kind="ExternalInput").ap(),
        "wq": nc.dram_tensor("wq", [C, CH], DT, kind="ExternalInput").ap(),
        "wk": nc.dram_tensor("wk", [C, CH], DT, kind="ExternalInput").ap(),
        "wv": nc.dram_tensor("wv", [C, CH], DT, kind="ExternalInput").ap(),
        "wo": nc.dram_tensor("wo", [CH, C], DT, kind="ExternalInput").ap(),
        "bias": nc.dram_tensor("bias", [CH, 2], mybir.dt.float32, kind="ExternalInput").ap(),
        "out": nc.dram_tensor("out", [T, C], DT, kind="ExternalOutput").ap(),
    }
    with tile.TileContext(nc) as tc:
        emit(tc, aps)
    nc.compile()
    return nc


_NC = None


def make_in_maps(query, key, value, Wq, bq, Wk, bk, Wv, bv, Wo, bo):
    query, key, value, Wq, bq, Wk, bk, Wv, bv, Wo, bo = (
        np.asarray(a, dtype=np.float32)
        for a in (query, key, value, Wq, bq, Wk, bk, Wv, bv, Wo, bo)
    )
    xq = np.ascontiguousarray(query.reshape(T, C).T).astype(NPDT)
    xk = np.ascontiguousarray(key.reshape(T, C).T).astype(NPDT)
    xv = np.ascontiguousarray(value.reshape(T, C).T).astype(NPDT)
    in_maps = []
    for c in range(NCORES):
        r = slice(CH * c, CH * (c + 1))
        in_maps.append({
            "xq": xq, "xk": xk, "xv": xv,
            "wq": np.ascontiguousarray(Wq[r, :].T).astype(NPDT),
            "wk": np.ascontiguousarray(Wk[r, :].T).astype(NPDT),
            "wv": np.ascontiguousarray(Wv[r, :].T).astype(NPDT),
            "wo": np.ascontiguousarray(Wo[:, r].T).astype(NPDT),
            "bias": np.ascontiguousarray(
                np.stack([bq[r], bk[r]], axis=1).astype(np.float32)),
        })
    return in_maps


def finish(partials, Wv_bias_args):
    Wo, bv, bo = Wv_bias_args
    out = np.zeros((T, C), np.float64)
    for p in partials:
        out += p.astype(np.float64)
    out += (np.asarray(Wo, np.float64) @ np.asarray(bv, np.float64)) + np.asarray(bo, np.float64)
    return out.astype(np.float32).reshape(B, N, C)


def kernel(query, key, value, Wq, bq, Wk, bk, Wv, bv, Wo, bo,
           _trace=False, _return_results=False):
    global _NC
    if _NC is None:
        _NC = build()
    in_maps = make_in_maps(query, key, value, Wq, bq, Wk, bk, Wv, bv, Wo, bo)
    res = run_bass_kernel_spmd(_NC, in_maps, core_ids=list(range(NCORES)), trace=_trace)
    out = finish([r["out"] for r in res.results], (Wo, bv, bo))
    if _return_results:
        return out, res
    return out


# revision 20
# speedup vs baseline: 1.0519x; 1.0519x over previous
"""CrossAttention kernel for 8 TRN2 NeuronCores.

Sharding (Megatron head-parallel): core c owns heads {2c, 2c+1} = output
channels [128c, 128c+128).
  - column-parallel q/k/v projections (full activations in, per-core head
    channels out)
  - full attention for the core's heads (both batch elements)
  - row-parallel out projection -> partial [4096, 1024] fp32; host sums the
    8 partials. v-bias and out-bias fold exactly into the host-side
    epilogue: out = sum_c partial_c + Wo @ bv + bo (softmax rows sum to 1).

Per-core dataflow (t = b*2048 + n, 4096 tokens):
  pass0 (per 1024-token chunk): pj [128ch, 512] = W @ xT chains, evicted in
    ONE [128,512] tensor_scalar_add to qT/kT [128ch, T] (rows 0:64 = head A
    d-dims, 64:128 = head B -- the natural layout, no duplication).
    v [t, ch] chains -> v_aug[u] [128 j, 16 jt, 130] (cols 65h+64 are ones).
  attention block (u, iw in 0..3, BOTH heads, i-window 512): 16 j-steps:
    scores: sp[:, 0:512]   = kT[0:64, j].T   @ qT[0:64, iw]    (head A)
            sp[:, 512:1024]= kT[64:128, j].T @ qT[64:128, iw]  (head B)
            -- the two matmuls use disjoint PE row groups, run concurrently
    exp:    pexp [128, 1024] = exp(0.125 * sp)   (ACT, PSUM->SBUF fp16)
    ctx:    cps[:, 0:512]   += v_aug[., jt, 0:65].T   @ pexp[:, 0:512]
            cps[:, 512:1024]+= v_aug[., jt, 65:130].T @ pexp[:, 512:1024]
            (row 64 of each half accumulates the softmax denominator r[i])
  normalize: one DVE copy cps->ctx_s [65,1024]; reciprocal_approx_fast on
    row 64; gpsimd partition_broadcast; 2 DVE muls -> ctxT[u] fp16.
  outproj: po [128t, 512] = ctxT[u][:, tt].T @ wo half; evict fp16, DMA out.

Projection / out-proj work units are interleaved between attention j-steps
so the PE fills the slack under the ACT-bound softmax exp stream.
PSUM banks: sps0/sps1 [128,1024] (2+2), cps [65,1024] (2), pj/po [128,512]
(1+1) = 8.
"""

import numpy as np
from contextlib import ExitStack

import concourse.bass as bass
import concourse.tile as tile
from concourse import bacc, mybir
from concourse.bass_utils import run_bass_kernel_spmd

AF = mybir.ActivationFunctionType

# ---- problem constants (hardcoded per contract) ----
B, N, C, H, D = 2, 2048, 1024, 16, 64
T = B * N            # 4096 flattened tokens
CH = 128             # channels per core = 2 heads * 64
NCORES = 8
SCALE = D ** -0.5    # 0.125

# ---- tunables ----
DT = mybir.dt.float16      # on-chip matmul/storage dtype
NPDT = np.float16
TCH = 1024                 # pass0 t-chunk
NTCH = T // TCH            # 4
FT = C // 128              # 8 f-tiles (contraction tiles for projections)
IW = 512                   # attention i-window per block
JT = 128                   # j tile (kv) size
NJT = N // JT              # 16


def emit(tc: tile.TileContext, aps: dict):
    nc = tc.nc
    ctx = ExitStack()
    with ctx:
        const = ctx.enter_context(tc.tile_pool(name="const", bufs=1))
        persist = ctx.enter_context(tc.tile_pool(name="persist", bufs=1))
        xpool = ctx.enter_context(tc.tile_pool(name="xpool", bufs=2))
        ppool = ctx.enter_context(tc.tile_pool(name="ppool", bufs=2))
        rpool = ctx.enter_context(tc.tile_pool(name="rpool", bufs=2))
        opool = ctx.enter_context(tc.tile_pool(name="opool", bufs=4))
        psum = ctx.enter_context(tc.tile_pool(name="psum", bufs=1, space="PSUM"))

        # ---- load weights (wq first: the q chain of chunk 0 runs first) ----
        w_sb = {name: const.tile([128, FT, CH], DT, name=name, tag=name)
                for name in ("wq", "wk", "wv")}
        wo_sb = const.tile([128, C], DT, tag="wo")
        bias_sb = const.tile([128, 2], mybir.dt.float32, tag="bias")

        def load_weights(names):
            # weights go on the ACT dispatch queue (hwdge) so the SP queue
            # starts streaming activations immediately; the rings interleave
            # the transfers and these are small
            for name in names:
                nc.scalar.dma_start(w_sb[name][:],
                                    aps[name].rearrange("(f p) m -> p f m", p=128))

        # ---- persistent activations ----
        # qT/kT [128 ch, T]: rows 0:64 head A dims, rows 64:128 head B
        qT = persist.tile([128, T], DT, name="qT", tag="qT")
        kT = persist.tile([128, T], DT, name="kT", tag="kT")
        # v_aug[u]: [128 j, 16 jt, 130]; cols 65h+64 are ones
        v_aug = [persist.tile([128, NJT, 130], DT, name=f"vaug{u}", tag=f"vaug{u}")
                 for u in range(B)]
        for u in range(B):
            for h in range(2):
                nc.vector.memset(v_aug[u][:, :, 65 * h + 64:65 * h + 65], 1.0)
        # ctxT[u]: [128 ch, 2048 i] normalized context^T (head B rows 64-127)
        ctxT = [persist.tile([128, N], DT, name=f"ctxT{u}", tag=f"ctxT{u}")
                for u in range(B)]

        xr = {k: aps[k].rearrange("(f p) t -> p f t", p=128) for k in ("xq", "xk", "xv")}

        # pass0 / outproj share the two 1-bank psum slots pj & po; pass0 is
        # done (chunk 3 consumed during block 3) before outproj starts
        # (block 4), so phases never contend.
        pingpong = [0]

        def small_ps(name):
            tag = ("pj", "po")[pingpong[0] % 2]
            pingpong[0] += 1
            return psum.tile([128, 512], mybir.dt.float32, name=name, tag=tag)

        # ---------------- pass0: projections for one 1024-token chunk -----
        # returns (dma_fn, [compute units]); the driver kicks the DMA a
        # block ahead of when the compute units get filled in.
        def pass0_units(tch):
            gen = []
            ts = slice(tch * TCH, (tch + 1) * TCH)
            xq_t = xpool.tile([128, FT, TCH], DT, tag="xq")
            xk_t = xpool.tile([128, FT, TCH], DT, tag="xk")
            xv_t = xpool.tile([128, FT, TCH], DT, tag="xv")

            order = ([("xq", xq_t), ("xk", xk_t), ("xv", xv_t)] if tch == 0
                     else [("xk", xk_t), ("xv", xv_t), ("xq", xq_t)])

            def u_dma():
                if tch == 0:
                    load_weights(["wq", "wk", "wv"])
                    nc.scalar.dma_start(wo_sb[:], aps["wo"])
                for nm, t in order:
                    nc.sync.dma_start(t[:], xr[nm][:, :, ts])

            def qk_half(x_t, wname, bcol, dst, half):
                # two sub-units sharing one psum accumulator: finer fill
                # granularity keeps per-jt PE slack small
                hs = slice(half * 512, (half + 1) * 512)
                gts = slice(tch * TCH + half * 512,
                            tch * TCH + half * 512 + 512)
                state = {}

                def unit_a():
                    pj = small_ps(f"c{tch}{wname}{half}")
                    state["pj"] = pj
                    for ft in range(4):
                        nc.tensor.matmul(pj[:], w_sb[wname][:, ft],
                                         x_t[:, ft, hs],
                                         start=(ft == 0), stop=False)

                def unit_b():
                    pj = state["pj"]
                    for ft in range(4, FT):
                        nc.tensor.matmul(pj[:], w_sb[wname][:, ft],
                                         x_t[:, ft, hs],
                                         start=False, stop=(ft == FT - 1))
                    nc.vector.tensor_scalar_add(
                        dst[:, gts], pj[:], bias_sb[:, bcol:bcol + 1])
                return [unit_a, unit_b]

            def v_half(vg, t4pair):
                # one 512-col psum with two t4 sub-chains (t4pair=0 -> t4 0,1)
                def unit():
                    pv = small_ps(f"c{tch}v{vg}{t4pair}")
                    pv4 = pv[:, 0:256].rearrange("p (t4 hh d) -> p t4 hh d",
                                                 t4=2, d=64)
                    for t4i in range(2):
                        t4 = t4pair * 2 + t4i
                        cs = slice(t4i * 128, (t4i + 1) * 128)
                        gcs = slice(vg * 512 + t4 * 128, vg * 512 + t4 * 128 + 128)
                        for ft in range(FT):
                            nc.tensor.matmul(pv[:, cs], xv_t[:, ft, gcs],
                                             w_sb["wv"][:, ft],
                                             start=(ft == 0), stop=(ft == FT - 1))
                    tt0 = tch * 8 + vg * 4 + t4pair * 2
                    u, jt0 = tt0 // 16, tt0 % 16
                    for h in range(2):
                        nc.vector.tensor_copy(
                            v_aug[u][:, jt0:jt0 + 2, 65 * h:65 * h + 64],
                            pv4[:, :, h])
                return unit

            q_units = [un for hf in range(2)
                       for un in qk_half(xq_t, "wq", 0, qT, hf)]
            k_units = [un for hf in range(2)
                       for un in qk_half(xk_t, "wk", 1, kT, hf)]
            v_units = [v_half(vg, tp) for vg in range(2) for tp in range(2)]
            if tch == 0:
                gen.extend(q_units + k_units + v_units)
            else:
                # k/v first: later j-steps of in-flight blocks need them
                gen.extend(k_units + v_units + q_units)
            return u_dma, gen

        # ---------------- out-proj for one 128-token tile ------------------
        oev = [0]

        def outproj_unit(u, tt):
            def unit():
                osb = opool.tile([128, C], DT, tag="osb")
                for oc in range(2):
                    po = small_ps(f"o{u}t{tt}{oc}")
                    nc.tensor.matmul(
                        po[:], ctxT[u][:, tt * 128:(tt + 1) * 128],
                        wo_sb[:, oc * 512:(oc + 1) * 512],
                        start=True, stop=True)
                    nc.vector.tensor_copy(osb[:, oc * 512:(oc + 1) * 512],
                                          po[:])
                    oev[0] += 1
                nc.sync.dma_start(
                    aps["out"][u * N + tt * 128:u * N + (tt + 1) * 128, :],
                    osb[:])
            return unit

        # ------------- attention block (u, iw): both heads, 16 j-steps ----
        def attention_block(u, iw, fill, last=False):
            i0 = u * N + iw * IW
            cps = psum.tile([65, 1024], mybir.dt.float32, name=f"cps{u}{iw}",
                            tag="cps")
            sps = [None, None]
            pexp = [None, None]

            def scores(jt):
                j0 = u * N + jt * JT
                sp = psum.tile([128, 1024], mybir.dt.float32,
                               name=f"sps{jt % 2}", tag=f"sps{jt % 2}")
                nc.tensor.matmul(sp[:, 0:512], kT[0:64, j0:j0 + JT],
                                 qT[0:64, i0:i0 + IW], start=True, stop=True)
                nc.tensor.matmul(sp[:, 512:1024], kT[64:128, j0:j0 + JT],
                                 qT[64:128, i0:i0 + IW], start=True, stop=True)
                sps[jt % 2] = sp

            def expstep(jt):
                pe = ppool.tile([128, 1024], DT, tag=f"pexp{jt % 2}")
                nc.scalar.activation(pe[:], sps[jt % 2][:], AF.Exp, scale=SCALE)
                pexp[jt % 2] = pe

            def ctxstep(jt):
                pe = pexp[jt % 2]
                st = dict(start=(jt == 0), stop=(jt == NJT - 1))
                nc.tensor.matmul(cps[:, 0:512], v_aug[u][:, jt, 0:65],
                                 pe[:, 0:512], **st)
                nc.tensor.matmul(cps[:, 512:1024], v_aug[u][:, jt, 65:130],
                                 pe[:, 512:1024], **st)

            scores(0)
            expstep(0)
            for jt in range(NJT):
                if jt + 1 < NJT:
                    scores(jt + 1)
                    expstep(jt + 1)
                ctxstep(jt)
                # no fills near the block boundary: lets the DVE queue
                # drain so the ctx_s eviction frees cps without delay
                if jt < NJT - 2:
                    fill()
            # normalize: one DVE copy frees the psum accumulator (next
            # block's ctx j-step 0 reuses it); then pull the denominator
            # row to a partition-0 tile with a native DVE copy (custom-ucode
            # ops ignore input partition offsets on HW!), 1/r (fast approx),
            # broadcast, scale. All off the ACT queue: ACT is the exp-stream
            # bottleneck.
            ctx_s = rpool.tile([65, 1024], mybir.dt.float32, tag="ctx_s")
            nc.vector.tensor_copy(ctx_s[:], cps[:])
            rrow = rpool.tile([1, 1024], mybir.dt.float32, name="rrow",
                              tag="rrow")
            if last:
                # tail latency: the exp stream is over, ACT is idle — pull
                # the denominator row in parallel with the DVE ctx copy
                nc.scalar.copy(rrow[:], cps[64:65, :])
            else:
                nc.vector.tensor_copy(rrow[:], ctx_s[64:65, :])
            rinv = rpool.tile([1, 1024], mybir.dt.float32, name="rinv",
                              tag="rinv")
            nc.vector.reciprocal_approx_fast(out=rinv[:], in_=rrow[:])
            rb = rpool.tile([64, 1024], mybir.dt.float32, name="rb", tag="rb")
            nc.gpsimd.partition_broadcast(rb[:], rinv[:])
            for h in range(2):
                nc.vector.tensor_mul(
                    ctxT[u][64 * h:64 * h + 64, iw * IW:(iw + 1) * IW],
                    ctx_s[0:64, 512 * h:512 * h + 512],
                    rb[:, 512 * h:512 * h + 512])

        # ---------------- driver ------------------------------------------
        # chunk 0: weights on the ACT queue + chunk-0/1 x streams upfront;
        # chunk-0 compute runs inline. Chunk DMAs for c2/c3 are kicked a
        # block ahead of their compute units.
        nc.scalar.dma_start(bias_sb[:], aps["bias"])
        dma1, units1 = pass0_units(1)
        dma2, units2 = pass0_units(2)
        dma3, units3 = pass0_units(3)
        dma0, units0 = pass0_units(0)
        dma0()
        dma1()
        for unit in units0:
            unit()

        # fill schedule: one list of 14 slots (j-steps 0..13) per block,
        # placed so a unit's slot lands at/after its input DMA eta — a
        # DMA-gated fill at the head of the in-order PE queue stalls the
        # whole attention stream. units order for tch>0: k(4) v(4) q(4).
        def grp(units, a, b):
            return list(units[a:b])

        NO = [None]
        op0 = [outproj_unit(0, tt) for tt in range(16)]
        op1 = [outproj_unit(1, tt) for tt in range(16)]
        u1k, u1v = grp(units1, 0, 4), grp(units1, 4, 8)
        u3k, u3v, u3q = grp(units3, 0, 4), grp(units3, 4, 8), grp(units3, 8, 12)
        # Balance pass0 + out-proj fills evenly across ALL blocks (~8 units
        # each) — cramming pass0 into blocks 0-3 makes them PE-bound at
        # ~1.7us/jt while blocks 4-7 idle the PE under the ACT exp stream.
        # Chunk-3 (u1's second half) legally slides INTO block 4 using the
        # same in-block gating as block 0: k half j8-11 before scores(8)
        # (slot<=6), k half j12-15 before scores(12) (slot<=10), v unit for
        # j-tiles 2i..2i+1 before ctx(2i) (slot<=2i-1).
        sched = [
            # b0: chunk-1 k/v feed THIS block's j-steps
            NO * 3 + [u1k[0], u1k[1], u1k[2], u1k[3],
                      u1v[0], None, u1v[1], None, u1v[2], None, u1v[3]],
            # b1: q1 ready early; k2 arrives mid-block
            grp(units1, 8, 12) + NO + grp(units2, 0, 4) + NO * 5,
            # b2: v2, q2
            grp(units2, 4, 8) + NO + grp(units2, 8, 12) + NO * 5,
            # b3: u0 out-proj (tokens 0:1024 normalized after b1)
            op0[0:8] + NO * 6,
            # b4: chunk-3 k/v gate this block's own j-steps
            [u3k[0], u3k[1], u3k[2], u3k[3], None, None, None,
             u3v[0], None, u3v[1], None, u3v[2], None, u3v[3]],
            # b5: q3 (feeds b6/b7 score windows) + u0 out-proj tail
            u3q + NO + op0[8:12] + NO * 5,
            op0[12:16] + NO + op1[0:4] + NO * 5,
            op1[4:12] + NO * 6,
        ]

        blocks = [(0, 0), (0, 1), (0, 2), (0, 3),
                  (1, 0), (1, 1), (1, 2), (1, 3)]
        for bi, (u, iw) in enumerate(blocks):
            if bi == 1:
                dma2()
            elif bi == 2:
                dma3()
            slots = list(sched[bi])

            def fill(slots=slots):
                if slots:
                    un = slots.pop(0)
                    if un is not None:
                        un()
            attention_block(u, iw, fill, last=(bi == 7))
        # tail out-proj: attention psum banks are free now — use the big
        # [128,1024] slots, and alternate evictions between the now-idle
        # ACT engine and DVE so consecutive tiles fully pipeline
        for i, tt in enumerate(range(12, N // 128)):
            ops = psum.tile([128, 1024], mybir.dt.float32, name=f"ot{tt}",
                            tag=f"sps{i % 2}")
            for oc in range(2):
                nc.tensor.matmul(
                    ops[:, oc * 512:(oc + 1) * 512],
                    ctxT[1][:, tt * 128:(tt + 1) * 128],
                    wo_sb[:, oc * 512:(oc + 1) * 512],
                    start=True, stop=True)
            osb = opool.tile([128, C], DT, tag="osb")
            if i % 2 == 0:
                nc.scalar.copy(osb[:], ops[:])
            else:
                nc.vector.tensor_copy(osb[:], ops[:])
            nc.sync.dma_start(
                aps["out"][N + tt * 128:N + (tt + 1) * 128, :], osb[:])


def build():
    nc = bacc.Bacc("TRN2", target_bir_lowering=False, debug=False)
    aps = {
        "xq": nc.dram_tensor("xq", [C, T], DT, kind="ExternalInput").ap(),
        "xk": nc.dram_tensor("xk", [C, T], DT, kind="ExternalInput").ap(),
        "xv": nc.dram_tensor("xv", [C, T], DT, kind="ExternalInput").ap(),
        "wq": nc.dram_tensor("wq", [C, CH], DT, kind="ExternalInput").ap(),
        "wk": nc.dram_tensor("wk", [C, CH], DT, kind="ExternalInput").ap(),
        "wv": nc.dram_tensor("wv", [C, CH], DT, kind="ExternalInput").ap(),
        "wo": nc.dram_tensor("wo", [CH, C], DT, kind="ExternalInput").ap(),
        "bias": nc.dram_tensor("bias", [CH, 2], mybir.dt.float32, kind="ExternalInput").ap(),
        "out": nc.dram_tensor("out", [T, C], DT, kind="ExternalOutput").ap(),
    }
    with tile.TileContext(nc) as tc:
        emit(tc, aps)
    nc.compile()
    return nc


_NC = None


def make_in_maps(query, key, value, Wq, bq, Wk, bk, Wv, bv, Wo, bo):
    query, key, value, Wq, bq, Wk, bk, Wv, bv, Wo, bo = (
        np.asarray(a, dtype=np.float32)
        for a in (query, key, value, Wq, bq, Wk, bk, Wv, bv, Wo, bo)
    )
    xq = np.ascontiguousarray(query.reshape(T, C).T).astype(NPDT)
    xk = np.ascontiguousarray(key.reshape(T, C).T).astype(NPDT)
    xv = np.ascontiguousarray(value.reshape(T, C).T).astype(NPDT)
    in_maps = []
    for c in range(NCORES):
        r = slice(CH * c, CH * (c + 1))
        in_maps.append({
            "xq": xq, "xk": xk, "xv": xv,
            "wq": np.ascontiguousarray(Wq[r, :].T).astype(NPDT),
            "wk": np.ascontiguousarray(Wk[r, :].T).astype(NPDT),
            "wv": np.ascontiguousarray(Wv[r, :].T).astype(NPDT),
            "wo": np.ascontiguousarray(Wo[:, r].T).astype(NPDT),
            "bias": np.ascontiguousarray(
                np.stack([bq[r], bk[r]], axis=1).astype(np.float32)),
        })
    return in_maps


def finish(partials, Wv_bias_args):
    Wo, bv, bo = Wv_bias_args
    out = np.zeros((T, C), np.float64)
    for p in partials:
        out += p.astype(np.float64)
    out += (np.asarray(Wo, np.float64) @ np.asarray(bv, np.float64)) + np.asarray(bo, np.float64)
    return out.astype(np.float32).reshape(B, N, C)


def kernel(query, key, value, Wq, bq, Wk, bk, Wv, bv, Wo, bo,
           _trace=False, _return_results=False):
    global _NC
    if _NC is None:
        _NC = build()
    in_maps = make_in_maps(query, key, value, Wq, bq, Wk, bk, Wv, bv, Wo, bo)
    res = run_bass_kernel_spmd(_NC, in_maps, core_ids=list(range(NCORES)), trace=_trace)
    out = finish([r["out"] for r in res.results], (Wo, bv, bo))
    if _return_results:
        return out, res
    return out


# revision 33
# speedup vs baseline: 1.0583x; 1.0060x over previous
"""CrossAttention kernel for 8 TRN2 NeuronCores.

Sharding (Megatron head-parallel): core c owns heads {2c, 2c+1} = output
channels [128c, 128c+128).
  - column-parallel q/k/v projections (full activations in, per-core head
    channels out)
  - full attention for the core's heads (both batch elements)
  - row-parallel out projection -> partial [4096, 1024] fp32; host sums the
    8 partials. v-bias and out-bias fold exactly into the host-side
    epilogue: out = sum_c partial_c + Wo @ bv + bo (softmax rows sum to 1).

Per-core dataflow (t = b*2048 + n, 4096 tokens):
  pass0 (per 1024-token chunk): pj [128ch, 512] = W @ xT chains, evicted in
    ONE [128,512] tensor_scalar_add to qT/kT [128ch, T] (rows 0:64 = head A
    d-dims, 64:128 = head B -- the natural layout, no duplication).
    v [t, ch] chains -> v_aug[u] [128 j, 16 jt, 130] (cols 65h+64 are ones).
  attention block (u, iw in 0..3, BOTH heads, i-window 512): 16 j-steps:
    scores: sp[:, 0:512]   = kT[0:64, j].T   @ qT[0:64, iw]    (head A)
            sp[:, 512:1024]= kT[64:128, j].T @ qT[64:128, iw]  (head B)
            -- the two matmuls use disjoint PE row groups, run concurrently
    exp:    pexp [128, 1024] = exp(0.125 * sp)   (ACT, PSUM->SBUF fp16)
    ctx:    cps[:, 0:512]   += v_aug[., jt, 0:65].T   @ pexp[:, 0:512]
            cps[:, 512:1024]+= v_aug[., jt, 65:130].T @ pexp[:, 512:1024]
            (row 64 of each half accumulates the softmax denominator r[i])
  normalize: one DVE copy cps->ctx_s [65,1024]; reciprocal_approx_fast on
    row 64; gpsimd partition_broadcast; 2 DVE muls -> ctxT[u] fp16.
  outproj: po [128t, 512] = ctxT[u][:, tt].T @ wo half; evict fp16, DMA out.

Projection / out-proj work units are interleaved between attention j-steps
so the PE fills the slack under the ACT-bound softmax exp stream.
PSUM banks: sps0/sps1 [128,1024] (2+2), cps [65,1024] (2), pj/po [128,512]
(1+1) = 8.
"""

import numpy as np
from contextlib import ExitStack

import concourse.bass as bass
import concourse.tile as tile
from concourse import bacc, mybir
from concourse.bass_utils import run_bass_kernel_spmd

AF = mybir.ActivationFunctionType

# ---- problem constants (hardcoded per contract) ----
B, N, C, H, D = 2, 2048, 1024, 16, 64
T = B * N            # 4096 flattened tokens
CH = 128             # channels per core = 2 heads * 64
NCORES = 8
SCALE = D ** -0.5    # 0.125

# ---- tunables ----
DT = mybir.dt.float16      # on-chip matmul/storage dtype
NPDT = np.float16
TCH = 1024                 # pass0 t-chunk
NTCH = T // TCH            # 4
FT = C // 128              # 8 f-tiles (contraction tiles for projections)
IW = 512                   # attention i-window per block
JT = 128                   # j tile (kv) size
NJT = N // JT              # 16


def emit(tc: tile.TileContext, aps: dict):
    nc = tc.nc
    ctx = ExitStack()
    with ctx:
        const = ctx.enter_context(tc.tile_pool(name="const", bufs=1))
        persist = ctx.enter_context(tc.tile_pool(name="persist", bufs=1))
        xpool = ctx.enter_context(tc.tile_pool(name="xpool", bufs=2))
        ppool = ctx.enter_context(tc.tile_pool(name="ppool", bufs=2))
        rpool = ctx.enter_context(tc.tile_pool(name="rpool", bufs=2))
        opool = ctx.enter_context(tc.tile_pool(name="opool", bufs=4))
        psum = ctx.enter_context(tc.tile_pool(name="psum", bufs=1, space="PSUM"))

        # ---- load weights (wq first: the q chain of chunk 0 runs first) ----
        w_sb = {name: const.tile([128, FT, CH], DT, name=name, tag=name)
                for name in ("wq", "wk", "wv")}
        wo_sb = const.tile([128, C], DT, tag="wo")
        bias_sb = const.tile([128, 2], mybir.dt.float32, tag="bias")

        def load_weights(names):
            # weights go on the ACT dispatch queue (hwdge) so the SP queue
            # starts streaming activations immediately; host pre-permutes
            # them to [128, FT, CH] so each partition line is contiguous
            # 2KB (256B lines triple the dispatch+transfer cost)
            for name in names:
                nc.scalar.dma_start(w_sb[name][:], aps[name])

        # ---- persistent activations ----
        # qT/kT [128 ch, T]: rows 0:64 head A dims, rows 64:128 head B
        qT = persist.tile([128, T], DT, name="qT", tag="qT")
        kT = persist.tile([128, T], DT, name="kT", tag="kT")
        # v_aug[u]: [128 j, 16 jt, 130]; cols 65h+64 are ones
        v_aug = [persist.tile([128, NJT, 130], DT, name=f"vaug{u}", tag=f"vaug{u}")
                 for u in range(B)]
        for u in range(B):
            for h in range(2):
                nc.vector.memset(v_aug[u][:, :, 65 * h + 64:65 * h + 65], 1.0)
        # ctxT[u]: [128 ch, 2048 i] normalized context^T (head B rows 64-127)
        ctxT = [persist.tile([128, N], DT, name=f"ctxT{u}", tag=f"ctxT{u}")
                for u in range(B)]

        xr = {k: aps[k].rearrange("(f p) t -> p f t", p=128) for k in ("xq", "xk", "xv")}

        # pass0 / outproj share the two 1-bank psum slots pj & po; pass0 is
        # done (chunk 3 consumed during block 3) before outproj starts
        # (block 4), so phases never contend.
        pingpong = [0]

        def small_ps(name):
            tag = ("pj", "po")[pingpong[0] % 2]
            pingpong[0] += 1
            return psum.tile([128, 512], mybir.dt.float32, name=name, tag=tag)

        # ---------------- pass0: projections for one 1024-token chunk -----
        # returns (dma_fn, [compute units]); the driver kicks the DMA a
        # block ahead of when the compute units get filled in.
        def pass0_units(tch):
            gen = []
            ts = slice(tch * TCH, (tch + 1) * TCH)
            xq_t = xpool.tile([128, FT, TCH], DT, tag="xq")
            xk_t = xpool.tile([128, FT, TCH], DT, tag="xk")
            xv_t = xpool.tile([128, FT, TCH], DT, tag="xv")

            order = ([("xq", xq_t), ("xk", xk_t), ("xv", xv_t)] if tch == 0
                     else [("xk", xk_t), ("xv", xv_t), ("xq", xq_t)])

            def u_dma():
                if tch == 0:
                    load_weights(["wq", "wk", "wv"])
                    nc.scalar.dma_start(wo_sb[:], aps["wo"])
                    # chunk 0 streams in token-halves: q/k/v half-0 first so
                    # the inline chains + attention start earliest; q half-1
                    # last (only needed by block 1's i-window)
                    tiles = dict(order)
                    seq = [("xq", 0), ("xk", 0), ("xv", 0),
                           ("xk", 1), ("xv", 1), ("xq", 1)]
                    for nm, hf in seq:
                        hs = slice(hf * 512, (hf + 1) * 512)
                        nc.sync.dma_start(tiles[nm][:, :, hs],
                                          xr[nm][:, :, hs])
                else:
                    for nm, t in order:
                        nc.sync.dma_start(t[:], xr[nm][:, :, ts])

            def qk_half(x_t, wname, bcol, dst, half):
                # two sub-units sharing one psum accumulator: finer fill
                # granularity keeps per-jt PE slack small
                hs = slice(half * 512, (half + 1) * 512)
                gts = slice(tch * TCH + half * 512,
                            tch * TCH + half * 512 + 512)
                state = {}

                def unit_a():
                    pj = small_ps(f"c{tch}{wname}{half}")
                    state["pj"] = pj
                    for ft in range(4):
                        nc.tensor.matmul(pj[:], w_sb[wname][:, ft],
                                         x_t[:, ft, hs],
                                         start=(ft == 0), stop=False)

                def unit_b():
                    pj = state["pj"]
                    for ft in range(4, FT):
                        nc.tensor.matmul(pj[:], w_sb[wname][:, ft],
                                         x_t[:, ft, hs],
                                         start=False, stop=(ft == FT - 1))
                    nc.vector.tensor_scalar_add(
                        dst[:, gts], pj[:], bias_sb[:, bcol:bcol + 1])
                return [unit_a, unit_b]

            def v_half(vg, t4pair):
                # one 512-col psum with two t4 sub-chains (t4pair=0 -> t4 0,1)
                def unit():
                    pv = small_ps(f"c{tch}v{vg}{t4pair}")
                    pv4 = pv[:, 0:256].rearrange("p (t4 hh d) -> p t4 hh d",
                                                 t4=2, d=64)
                    for t4i in range(2):
                        t4 = t4pair * 2 + t4i
                        cs = slice(t4i * 128, (t4i + 1) * 128)
                        gcs = slice(vg * 512 + t4 * 128, vg * 512 + t4 * 128 + 128)
                        for ft in range(FT):
                            nc.tensor.matmul(pv[:, cs], xv_t[:, ft, gcs],
                                             w_sb["wv"][:, ft],
                                             start=(ft == 0), stop=(ft == FT - 1))
                    tt0 = tch * 8 + vg * 4 + t4pair * 2
                    u, jt0 = tt0 // 16, tt0 % 16
                    for h in range(2):
                        nc.vector.tensor_copy(
                            v_aug[u][:, jt0:jt0 + 2, 65 * h:65 * h + 64],
                            pv4[:, :, h])
                return unit

            q_units = [un for hf in range(2)
                       for un in qk_half(xq_t, "wq", 0, qT, hf)]
            k_units = [un for hf in range(2)
                       for un in qk_half(xk_t, "wk", 1, kT, hf)]
            v_units = [v_half(vg, tp) for vg in range(2) for tp in range(2)]
            if tch == 0:
                gen.extend(q_units + k_units + v_units)
            else:
                # k/v first: later j-steps of in-flight blocks need them
                gen.extend(k_units + v_units + q_units)
            return u_dma, gen

        # ---------------- out-proj for one 128-token tile ------------------
        oev = [0]

        def outproj_unit(u, tt):
            def unit():
                osb = opool.tile([128, C], DT, tag="osb")
                for oc in range(2):
                    po = small_ps(f"o{u}t{tt}{oc}")
                    nc.tensor.matmul(
                        po[:], ctxT[u][:, tt * 128:(tt + 1) * 128],
                        wo_sb[:, oc * 512:(oc + 1) * 512],
                        start=True, stop=True)
                    nc.vector.tensor_copy(osb[:, oc * 512:(oc + 1) * 512],
                                          po[:])
                    oev[0] += 1
                nc.sync.dma_start(
                    aps["out"][u * N + tt * 128:u * N + (tt + 1) * 128, :],
                    osb[:])
            return unit

        # ------------- attention block (u, iw): both heads, 16 j-steps ----
        def attention_block(u, iw, fill, last=False, nfill=NJT - 2):
            i0 = u * N + iw * IW
            cps = psum.tile([65, 1024], mybir.dt.float32, name=f"cps{u}{iw}",
                            tag="cps")
            sps = [None, None]
            pexp = [None, None]

            def scores(jt):
                j0 = u * N + jt * JT
                sp = psum.tile([128, 1024], mybir.dt.float32,
                               name=f"sps{jt % 2}", tag=f"sps{jt % 2}")
                nc.tensor.matmul(sp[:, 0:512], kT[0:64, j0:j0 + JT],
                                 qT[0:64, i0:i0 + IW], start=True, stop=True)
                nc.tensor.matmul(sp[:, 512:1024], kT[64:128, j0:j0 + JT],
                                 qT[64:128, i0:i0 + IW], start=True, stop=True)
                sps[jt % 2] = sp

            def expstep(jt):
                pe = ppool.tile([128, 1024], DT, tag=f"pexp{jt % 2}")
                nc.scalar.activation(pe[:], sps[jt % 2][:], AF.Exp, scale=SCALE)
                pexp[jt % 2] = pe

            def ctxstep(jt):
                pe = pexp[jt % 2]
                st = dict(start=(jt == 0), stop=(jt == NJT - 1))
                nc.tensor.matmul(cps[:, 0:512], v_aug[u][:, jt, 0:65],
                                 pe[:, 0:512], **st)
                nc.tensor.matmul(cps[:, 512:1024], v_aug[u][:, jt, 65:130],
                                 pe[:, 512:1024], **st)

            scores(0)
            expstep(0)
            for jt in range(NJT):
                if jt + 1 < NJT:
                    scores(jt + 1)
                    expstep(jt + 1)
                ctxstep(jt)
                # no fills near the block boundary: lets the DVE queue
                # drain so the ctx_s eviction frees cps without delay
                if jt < nfill:
                    fill()
            # normalize: one DVE copy frees the psum accumulator (next
            # block's ctx j-step 0 reuses it); then pull the denominator
            # row to a partition-0 tile with a native DVE copy (custom-ucode
            # ops ignore input partition offsets on HW!), 1/r (fast approx),
            # broadcast, scale. All off the ACT queue: ACT is the exp-stream
            # bottleneck.
            ctx_s = rpool.tile([65, 1024], mybir.dt.float32, tag="ctx_s")
            nc.vector.tensor_copy(ctx_s[:], cps[:])
            # last block: fresh tag (no pool WAR) + ACT copy (idle then) so
            # the denominator extraction runs in parallel with the DVE copy
            rrow = rpool.tile([1, 1024], mybir.dt.float32, name="rrow",
                              tag="rrowT" if last else "rrow")
            if last:
                nc.scalar.copy(rrow[:], cps[64:65, :])
            else:
                nc.vector.tensor_copy(rrow[:], ctx_s[64:65, :])
            rinv = rpool.tile([1, 1024], mybir.dt.float32, name="rinv",
                              tag="rinv")
            nc.vector.reciprocal_approx_fast(out=rinv[:], in_=rrow[:])
            rb = rpool.tile([64, 1024], mybir.dt.float32, name="rb", tag="rb")
            nc.gpsimd.partition_broadcast(rb[:], rinv[:])
            for h in range(2):
                nc.vector.tensor_mul(
                    ctxT[u][64 * h:64 * h + 64, iw * IW:(iw + 1) * IW],
                    ctx_s[0:64, 512 * h:512 * h + 512],
                    rb[:, 512 * h:512 * h + 512])

        # ---------------- driver ------------------------------------------
        # chunk 0: weights on the ACT queue + chunk-0/1 x streams upfront;
        # chunk-0 compute runs inline. Chunk DMAs for c2/c3 are kicked a
        # block ahead of their compute units.
        nc.scalar.dma_start(bias_sb[:], aps["bias"])
        dma1, units1 = pass0_units(1)
        dma2, units2 = pass0_units(2)
        dma3, units3 = pass0_units(3)
        dma0, units0 = pass0_units(0)
        dma0()
        dma1()
        # inline only what block 0's first j-steps need: q half-0, k half-0,
        # v tokens 0:256 (units0 order: q 0-3, k 4-7, v 8-11); the rest of
        # chunk 0 slides into block 0's fill slots at its DMA arrival times
        for unit in (units0[0], units0[1], units0[4], units0[5], units0[8]):
            unit()

        # fill schedule: one list of 14 slots (j-steps 0..13) per block,
        # placed so a unit's slot lands at/after its input DMA eta — a
        # DMA-gated fill at the head of the in-order PE queue stalls the
        # whole attention stream. units order for tch>0: k(4) v(4) q(4).
        def grp(units, a, b):
            return list(units[a:b])

        NO = [None]
        op0 = [outproj_unit(0, tt) for tt in range(16)]
        op1 = [outproj_unit(1, tt) for tt in range(16)]
        u1k, u1v = grp(units1, 0, 4), grp(units1, 4, 8)
        u3k, u3v, u3q = grp(units3, 0, 4), grp(units3, 4, 8), grp(units3, 8, 12)
        # Balance pass0 + out-proj fills evenly across ALL blocks (~8 units
        # each) — cramming pass0 into blocks 0-3 makes them PE-bound at
        # ~1.7us/jt while blocks 4-7 idle the PE under the ACT exp stream.
        # Chunk-3 (u1's second half) legally slides INTO block 4 using the
        # same in-block gating as block 0: k half j8-11 before scores(8)
        # (slot<=6), k half j12-15 before scores(12) (slot<=10), v unit for
        # j-tiles 2i..2i+1 before ctx(2i) (slot<=2i-1).
        sched = [
            # b0: rest of chunk 0 AND chunk 1's k/v feed THIS block's
            # j-steps; slotted at both their consumption deadlines and DMA
            # arrival order. The q half-1 PAIR must both land inside b0 —
            # block 1's first scores read qT[:,512:1024] at emission slot 0,
            # before any b1 fill runs (b0 gets a 15th fill slot for this).
            [units0[9], units0[6], units0[7], units0[10],
             u1k[0], units0[11], u1k[1], u1v[0], u1k[2], u1v[1],
             u1k[3], u1v[2], u1v[3], units0[2], units0[3]],
            # b1: chunk-1 q, chunk-2 k
            grp(units1, 8, 12) + grp(units2, 0, 4) + NO * 6,
            # b2: v2, q2
            grp(units2, 4, 8) + NO + grp(units2, 8, 12) + NO * 5,
            # b3: u0 out-proj (tokens 0:1024 normalized after b1)
            op0[0:8] + NO * 6,
            # b4: chunk-3 k/v gate this block's own j-steps
            [u3k[0], u3k[1], u3k[2], u3k[3], None, None, None,
             u3v[0], None, u3v[1], None, u3v[2], None, u3v[3]],
            # b5: q3 (feeds b6/b7 score windows) + u0 out-proj tail
            u3q + NO + op0[8:12] + NO * 5,
            op0[12:16] + NO + op1[0:4] + NO * 5,
            op1[4:12] + NO * 6,
        ]

        blocks = [(0, 0), (0, 1), (0, 2), (0, 3),
                  (1, 0), (1, 1), (1, 2), (1, 3)]
        for bi, (u, iw) in enumerate(blocks):
            if bi == 1:
                dma2()
            elif bi == 2:
                dma3()
            slots = list(sched[bi])

            def fill(slots=slots):
                if slots:
                    un = slots.pop(0)
                    if un is not None:
                        un()
            attention_block(u, iw, fill, last=(bi == 7),
                            nfill=(NJT - 1 if bi == 0 else NJT - 2))
        # keep the PE clock ramped through the final normalize: the p-state
        # drops back to 1.2GHz after an idle gap, making the tail's
        # out-proj matmuls ~1.6x slower. Harmless back-to-back matmuls into
        # a dead psum bank hold the ramp (results never read).
        warm = psum.tile([128, 1024], mybir.dt.float32, name="warm",
                         tag="sps1")
        for _ in range(26):
            nc.tensor.matmul(warm[:, 0:512], ctxT[1][:, 0:128],
                             wo_sb[:, 0:512], start=True, stop=True)
        # tail out-proj: attention psum banks are free now — use the big
        # [128,1024] slots, and alternate evictions between the now-idle
        # ACT engine and DVE so consecutive tiles fully pipeline
        for i, tt in enumerate(range(12, N // 128)):
            ops = psum.tile([128, 1024], mybir.dt.float32, name=f"ot{tt}",
                            tag=f"sps{i % 2}")
            for oc in range(2):
                nc.tensor.matmul(
                    ops[:, oc * 512:(oc + 1) * 512],
                    ctxT[1][:, tt * 128:(tt + 1) * 128],
                    wo_sb[:, oc * 512:(oc + 1) * 512],
                    start=True, stop=True)
            osb = opool.tile([128, C], DT, tag="osb")
            if i % 2 == 0:
                nc.scalar.copy(osb[:], ops[:])
            else:
                nc.vector.tensor_copy(osb[:], ops[:])
            nc.sync.dma_start(
                aps["out"][N + tt * 128:N + (tt + 1) * 128, :], osb[:])


def build():
    nc = bacc.Bacc("TRN2", target_bir_lowering=False, debug=False)
    aps = {
        "xq": nc.dram_tensor("xq", [C, T], DT, kind="ExternalInput").ap(),
        "xk": nc.dram_tensor("xk", [C, T], DT, kind="ExternalInput").ap(),
        "xv": nc.dram_tensor("xv", [C, T], DT, kind="ExternalInput").ap(),
        "wq": nc.dram_tensor("wq", [128, FT, CH], DT, kind="ExternalInput").ap(),
        "wk": nc.dram_tensor("wk", [128, FT, CH], DT, kind="ExternalInput").ap(),
        "wv": nc.dram_tensor("wv", [128, FT, CH], DT, kind="ExternalInput").ap(),
        "wo": nc.dram_tensor("wo", [CH, C], DT, kind="ExternalInput").ap(),
        "bias": nc.dram_tensor("bias", [CH, 2], mybir.dt.float32, kind="ExternalInput").ap(),
        "out": nc.dram_tensor("out", [T, C], DT, kind="ExternalOutput").ap(),
    }
    with tile.TileContext(nc) as tc:
        emit(tc, aps)
    nc.compile()
    return nc


_NC = None


def make_in_maps(query, key, value, Wq, bq, Wk, bk, Wv, bv, Wo, bo):
    query, key, value, Wq, bq, Wk, bk, Wv, bv, Wo, bo = (
        np.asarray(a, dtype=np.float32)
        for a in (query, key, value, Wq, bq, Wk, bk, Wv, bv, Wo, bo)
    )
    xq = np.ascontiguousarray(query.reshape(T, C).T).astype(NPDT)
    xk = np.ascontiguousarray(key.reshape(T, C).T).astype(NPDT)
    xv = np.ascontiguousarray(value.reshape(T, C).T).astype(NPDT)
    def wperm(W, r):
        # [C, CH] -> [128 p, FT f, CH m]: each SBUF partition line becomes
        # one contiguous 2KB DMA line instead of FT 256B strided lines
        a = W[r, :].T.reshape(FT, 128, CH).transpose(1, 0, 2)
        return np.ascontiguousarray(a).astype(NPDT)

    in_maps = []
    for c in range(NCORES):
        r = slice(CH * c, CH * (c + 1))
        in_maps.append({
            "xq": xq, "xk": xk, "xv": xv,
            "wq": wperm(Wq, r),
            "wk": wperm(Wk, r),
            "wv": wperm(Wv, r),
            "wo": np.ascontiguousarray(Wo[:, r].T).astype(NPDT),
            "bias": np.ascontiguousarray(
                np.stack([bq[r], bk[r]], axis=1).astype(np.float32)),
        })
    return in_maps


def finish(partials, Wv_bias_args):
    Wo, bv, bo = Wv_bias_args
    out = np.zeros((T, C), np.float64)
    for p in partials:
        out += p.astype(np.float64)
    out += (np.asarray(Wo, np.float64) @ np.asarray(bv, np.float64)) + np.asarray(bo, np.float64)
    return out.astype(np.float32).reshape(B, N, C)


def kernel(query, key, value, Wq, bq, Wk, bk, Wv, bv, Wo, bo,
           _trace=False, _return_results=False):
    global _NC
    if _NC is None:
        _NC = build()
    in_maps = make_in_maps(query, key, value, Wq, bq, Wk, bk, Wv, bv, Wo, bo)
    res = run_bass_kernel_spmd(_NC, in_maps, core_ids=list(range(NCORES)), trace=_trace)
    out = finish([r["out"] for r in res.results], (Wo, bv, bo))
    if _return_results:
        return out, res
    return out
